# revision 4
# baseline (speedup 1.0000x reference)
"""Multi-head attention TRN2 kernel (B=2, N=2048, D=1024, H=16).

Sharding: tensor-parallel over heads. Each of the 8 cores owns 2 heads
(both batch elements) end-to-end through QKV projection and attention,
then the per-head attention outputs are AllGathered (bf16, per batch
half) and each core computes a 128-column slice of the output
projection.

QKV + attention matmuls run in float32r (full PE rate for >=256-wide
outputs, ~1.5e-4 scaled error); the projection path (AllGather payload,
Wproj, projection matmul) runs in bf16 to halve the collective wire
time and the gathered-activation reload (~1-2e-3 scaled error, well
under the 2e-2 gate).

Softmax runs without max-subtraction (scores are O(5); exp is safe in
fp32): S^T is computed key-major via matmul(lhsT=kT, rhs=qT), exp'd on
ScalarE, and the denominator comes from a ones-column appended to V in
the P^T@V matmul. The S->exp->PV software pipeline is two steps deep so
ScalarE's exp latency stays off the PE critical path. PSUM is released
with a single [65 x IBLK] copy; the reciprocal + broadcast + normalize
run off the critical path (approx reciprocal on DVE, multiply on
GpSimd, DRAM broadcast round-trip on the sync queue).

Emission order interleaves phases (QKV b0 -> attention b0/ib0 -> QKV b1
-> remaining attention -> projection) so ScalarE's exp stream starts
~40us earlier and input DMA overlaps attention.

All host-side tensors are laid out so every DMA descriptor is >=2KB
contiguous per partition (the baseline's strided rearrange loads ran
the HBM queues at ~75GB/s; these run near peak).

Self-contained: hardcodes shapes from the problem spec.
"""

import sys

for _p in ("/opt/trn_rl_repo", "/root/.axon_site/_ro/trn_rl_repo"):
    if _p not in sys.path:
        sys.path.append(_p)

import numpy as np
from contextlib import ExitStack

import concourse.bass as bass
import concourse.tile as tile
from concourse import mybir, bacc
from concourse.bass_utils import run_bass_kernel_spmd

F32 = mybir.dt.float32
F32R = mybir.dt.float32r
BF16 = mybir.dt.bfloat16
EXP = mybir.ActivationFunctionType.Exp

B = 2
N = 2048
D = 1024
H = 16
DEPTH = 64
TOK = B * N            # 4096 tokens total (both batches)
KC = D // 128          # 8 contraction chunks of 128
NBLK = TOK // 512      # 8 token blocks for streaming projections
SCALE = 1.0 / np.sqrt(DEPTH)
NCORES = 8
IBLK = 1024            # query-block width in attention
NSUB = IBLK // 512     # matmuls per psum tile (N<=512 for 4-byte dtypes)
HALF = N // 2


def build_nc(qkv_dt=F32R, attn_dt=F32R, proj_dt=BF16):
    nc = bacc.Bacc(None)

    def dram_dt(dt):
        return F32 if dt == F32R else dt

    def cast(ap, dt):
        return ap.bitcast(F32R) if dt == F32R else ap

    # x^T stored block-major: [128, blk, kc, 512] so each 512-token block
    # is 16KB contiguous per partition.
    xt = nc.dram_tensor("xt", [128, NBLK, KC, 512], dram_dt(qkv_dt),
                        kind="ExternalInput")
    # weights pre-permuted to [128, kc, 128] (4KB contiguous/partition)
    wq = nc.dram_tensor("wq", [128, KC, 128], dram_dt(qkv_dt), kind="ExternalInput")
    wk = nc.dram_tensor("wk", [128, KC, 128], dram_dt(qkv_dt), kind="ExternalInput")
    wv = nc.dram_tensor("wv", [128, KC, 128], dram_dt(qkv_dt), kind="ExternalInput")
    wp = nc.dram_tensor("wp", [128, KC, 128], dram_dt(proj_dt), kind="ExternalInput")
    bq = nc.dram_tensor("bq", [128, 1], F32, kind="ExternalInput")
    bk = nc.dram_tensor("bk", [128, 1], F32, kind="ExternalInput")
    bv = nc.dram_tensor("bv", [128, 1], F32, kind="ExternalInput")
    bp = nc.dram_tensor("bp", [128, 1], F32, kind="ExternalInput")
    ident = nc.dram_tensor(
        "ident", [128, 128], dram_dt(attn_dt), kind="ExternalInput"
    )
    ones = nc.dram_tensor("ones", [128, 1], dram_dt(attn_dt), kind="ExternalInput")
    out = nc.dram_tensor("o", [128, TOK], F32, kind="ExternalOutput")

    # Collective staging (split per (batch, half) so early AllGathers
    # overlap later attention).
    ag_in = [
        [nc.dram_tensor(f"ag_in{b}_{h}", [128, HALF], proj_dt) for h in range(2)]
        for b in range(B)
    ]
    ag_out = [
        [
            nc.dram_tensor(f"ag_out{b}_{h}", [D, HALF], proj_dt,
                           addr_space="Shared")
            for h in range(2)
        ]
        for b in range(B)
    ]

    with tile.TileContext(nc) as tc, ExitStack() as ctx:
        wpool = ctx.enter_context(tc.tile_pool(name="w", bufs=1))
        qkpool = ctx.enter_context(tc.tile_pool(name="qk", bufs=1))
        vpool = ctx.enter_context(tc.tile_pool(name="v2", bufs=1))
        xpool = ctx.enter_context(tc.tile_pool(name="x", bufs=3))
        vtpool = ctx.enter_context(tc.tile_pool(name="vt", bufs=2))
        ptpool = ctx.enter_context(tc.tile_pool(name="pt", bufs=3))
        unpool = ctx.enter_context(tc.tile_pool(name="un", bufs=2))
        rpool = ctx.enter_context(tc.tile_pool(name="r", bufs=2))
        rdpool = ctx.enter_context(tc.tile_pool(name="rd", bufs=2, space="DRAM"))
        apool = ctx.enter_context(tc.tile_pool(name="ap", bufs=2))
        oupool = ctx.enter_context(tc.tile_pool(name="ou", bufs=2))
        # PSUM budget (8 banks of 2KB/partition):
        #   ps_one (bufs=1): po [65,1024]                -> 2 banks
        #   ps_mm (bufs=2): mmA [128,512] qkv/transpose/proj -> 2 banks
        #   ps_two (bufs=2): ss [128,1024]               -> 4 banks
        ps_one = ctx.enter_context(tc.tile_pool(name="ps1", bufs=1, space="PSUM"))
        ps_mm = ctx.enter_context(tc.tile_pool(name="psm", bufs=2, space="PSUM"))
        ps_two = ctx.enter_context(tc.tile_pool(name="ps2", bufs=2, space="PSUM"))

        # ---- weights / constants ----
        w_q = wpool.tile([128, KC, 128], qkv_dt, tag="w_q")
        w_k = wpool.tile([128, KC, 128], qkv_dt, tag="w_k")
        w_v = wpool.tile([128, KC, 128], qkv_dt, tag="w_v")
        w_p = wpool.tile([128, KC, 128], proj_dt, tag="w_p")
        # q weights first on the sync queue so the first matmul can start
        # as soon as xb0 lands; the rest stream in behind on gpsimd.
        nc.sync.dma_start(out=w_q, in_=cast(wq[:], qkv_dt))
        nc.sync.dma_start(out=w_k, in_=cast(wk[:], qkv_dt))
        nc.sync.dma_start(out=w_v, in_=cast(wv[:], qkv_dt))
        nc.gpsimd.dma_start(out=w_p, in_=cast(wp[:], proj_dt))
        b_q = wpool.tile([128, 1], F32, tag="b_q")
        b_k = wpool.tile([128, 1], F32, tag="b_k")
        b_v = wpool.tile([128, 1], F32, tag="b_v")
        b_p = wpool.tile([128, 1], F32, tag="b_p")
        for t, src in ((b_q, bq), (b_k, bk), (b_v, bv), (b_p, bp)):
            nc.gpsimd.dma_start(out=t, in_=src[:])
        id_t = wpool.tile([128, 128], attn_dt, tag="id_t")
        nc.gpsimd.dma_start(out=id_t, in_=cast(ident[:], attn_dt))

        # qT/kT: [feature 128 (= 2 heads x 64), token 4096]; head hl in rows
        # hl*64:(hl+1)*64 so both S^T operands share a partition base.
        qT = qkpool.tile([128, TOK], attn_dt, tag="qT")
        kT = qkpool.tile([128, TOK], attn_dt, tag="kT")
        # V2: [token part, 32 token-chunks, 130]: v_h0 | ones | v_h1 | ones
        V2 = vpool.tile([128, TOK // 128, 130], attn_dt, tag="V2")
        nc.gpsimd.dma_start(
            out=V2[:, :, 64:65],
            in_=cast(ones[:].to_broadcast((128, TOK // 128, 1)), attn_dt),
        )
        nc.gpsimd.dma_start(
            out=V2[:, :, 129:130],
            in_=cast(ones[:].to_broadcast((128, TOK // 128, 1)), attn_dt),
        )

        # ================= phase A: QKV projection for a block range =====
        def qkv_blocks(blo, bhi):
            for blk in range(blo, bhi):
                xb = xpool.tile([128, KC, 512], qkv_dt, tag="xb")
                eng = nc.sync if blk % 2 == 0 else nc.scalar
                eng.dma_start(out=xb, in_=cast(xt[:, blk], qkv_dt))
                for name, w_t, b_t in (
                    ("q", w_q, b_q), ("k", w_k, b_k), ("v", w_v, b_v)
                ):
                    t0 = blk * 512
                    ps = ps_mm.tile([128, 512], F32, tag="mmA")
                    for kc in range(KC):
                        nc.tensor.matmul(
                            out=ps,
                            lhsT=w_t[:, kc, :],
                            rhs=xb[:, kc, :],
                            start=(kc == 0),
                            stop=(kc == KC - 1),
                        )
                    if name == "q":
                        nc.vector.tensor_scalar_add(
                            out=qT[:, t0 : t0 + 512], in0=ps, scalar1=b_t
                        )
                    elif name == "k":
                        nc.vector.tensor_scalar_add(
                            out=kT[:, t0 : t0 + 512], in0=ps, scalar1=b_t
                        )
                    else:
                        vtmp = vtpool.tile([128, 512], attn_dt, tag="vtmp")
                        nc.vector.tensor_scalar_add(out=vtmp, in0=ps, scalar1=b_t)
                        # transpose 4x [128,128] -> V2 token chunks
                        for s in range(4):
                            ch = blk * 4 + s
                            ps_t = ps_mm.tile([128, 128], attn_dt, tag="mmA")
                            nc.tensor.transpose(
                                out=ps_t,
                                in_=vtmp[:, s * 128 : (s + 1) * 128],
                                identity=id_t,
                            )
                            nc.vector.tensor_copy(
                                out=V2[:, ch, 0:64], in_=ps_t[:, 0:64]
                            )
                            nc.vector.tensor_copy(
                                out=V2[:, ch, 65:129], in_=ps_t[:, 64:128]
                            )

        # ===== phase B: attention for one (batch, query-block) ==========
        def attn_chunk(b, ib):
            for hl in range(2):
                hs = hl * 64
                voff = hl * 65
                i0 = b * N + ib * IBLK
                ps_o = ps_one.tile([65, IBLK], F32, tag="po")
                NJC = N // 128

                def s_step(jc):
                    j0 = b * N + jc * 128
                    ps_s = ps_two.tile([128, IBLK], F32, tag="ss")
                    for su in range(NSUB):
                        nc.tensor.matmul(
                            out=ps_s[:, su * 512 : (su + 1) * 512],
                            lhsT=kT[hs : hs + 64, j0 : j0 + 128],
                            rhs=qT[
                                hs : hs + 64,
                                i0 + su * 512 : i0 + (su + 1) * 512,
                            ],
                            start=True,
                            stop=True,
                        )
                    pt = ptpool.tile([128, IBLK], attn_dt, tag="pt")
                    nc.scalar.activation(
                        out=pt, in_=ps_s, func=EXP, scale=float(SCALE)
                    )
                    return pt

                # two-deep software pipeline: exp(jc+1) and exp(jc+2) are
                # in flight while PV(jc) streams, so ScalarE latency never
                # stalls the PE.
                pt_q = [s_step(0), s_step(1)]
                for jc in range(NJC):
                    pt_cur = pt_q.pop(0)
                    if jc + 2 < NJC:
                        pt_q.append(s_step(jc + 2))
                    for su in range(NSUB):
                        nc.tensor.matmul(
                            out=ps_o[:, su * 512 : (su + 1) * 512],
                            lhsT=V2[:, ((b * N + jc * 128) // 128), voff : voff + 65],
                            rhs=pt_cur[:, su * 512 : (su + 1) * 512],
                            start=(jc == 0),
                            stop=(jc == NJC - 1),
                        )
                # single copy (rows + denominator) releases PSUM; the
                # normalize chain runs off the PE critical path.
                und = unpool.tile([65, IBLK], F32, tag="und")
                nc.vector.tensor_copy(out=und, in_=ps_o)
                rd = rdpool.tile([1, IBLK], F32, tag="rd")
                nc.sync.dma_start(out=rd, in_=und[64:65, :])
                rr = rpool.tile([64, IBLK], F32, tag="rr")
                nc.sync.dma_start(out=rr, in_=rd.to_broadcast((64, IBLK)))
                rcp = rpool.tile([64, IBLK], F32, tag="rcp")
                nc.vector.reciprocal_approx_fast(out=rcp, in_=rr)
                unr = unpool.tile([64, IBLK], proj_dt, tag="unr")
                nc.gpsimd.tensor_mul(out=unr, in0=und[0:64, :], in1=rcp)
                nc.sync.dma_start(out=ag_in[b][ib][hs : hs + 64, :], in_=unr)
            # both heads of (b, ib-half) staged: gather it
            nc.gpsimd.collective_compute(
                "AllGather",
                mybir.AluOpType.bypass,
                ins=[ag_in[b][ib][:]],
                outs=[ag_out[b][ib][:]],
                replica_groups=[list(range(NCORES))],
            )

        # ======= phase D: output projection (128 columns/core) =======
        def proj_half(b, hf):
            ag_r = ag_out[b][hf].rearrange("(kc p) t -> p kc t", p=128)
            ab = apool.tile([128, KC, HALF], proj_dt, tag="ab")
            nc.gpsimd.dma_start(out=ab, in_=ag_r)
            for i2 in range(HALF // 512):
                i0 = i2 * 512
                ps = ps_mm.tile([128, 512], F32, tag="mmA")
                for kc in range(KC):
                    nc.tensor.matmul(
                        out=ps,
                        lhsT=w_p[:, kc, :],
                        rhs=ab[:, kc, i0 : i0 + 512],
                        start=(kc == 0),
                        stop=(kc == KC - 1),
                    )
                ot = oupool.tile([128, 512], F32, tag="ot")
                nc.vector.tensor_scalar_add(out=ot, in0=ps, scalar1=b_p)
                to = b * N + hf * HALF + i0
                nc.sync.dma_start(out=out[:, to : to + 512], in_=ot)

        # ---- interleaved emission ----
        qkv_blocks(0, 4)       # b0 tokens
        attn_chunk(0, 0)       # attention b0 first half + AG
        qkv_blocks(4, 8)       # b1 tokens (DMA overlaps attention above)
        attn_chunk(0, 1)
        attn_chunk(1, 0)
        attn_chunk(1, 1)
        proj_half(0, 0)
        proj_half(0, 1)
        proj_half(1, 0)
        proj_half(1, 1)

    nc.compile()
    return nc


def np_dt(dt):
    return mybir.dt.np(F32 if dt == F32R else dt)


def prep_in_maps(x, Wqkv, bqkv, Wproj, bproj, qkv_dt=F32R, attn_dt=F32R,
                 proj_dt=BF16):
    x = np.asarray(x, dtype=np.float32)
    Wqkv = np.asarray(Wqkv, dtype=np.float32)
    bqkv = np.asarray(bqkv, dtype=np.float32)
    Wproj = np.asarray(Wproj, dtype=np.float32)
    bproj = np.asarray(bproj, dtype=np.float32)

    # x^T block-major: [128, blk, kc, 512]; row kc*128+p of x^T.
    xT = np.ascontiguousarray(x.reshape(TOK, D).T)     # [D, TOK]
    xtn = np.ascontiguousarray(
        xT.reshape(KC, 128, NBLK, 512).transpose(1, 2, 0, 3)
    ).astype(np_dt(qkv_dt))
    identity = np.eye(128, dtype=np_dt(attn_dt))
    ones_col = np.ones((128, 1), dtype=np_dt(attn_dt))

    def perm_w(w):  # [D, 128] -> [128, KC, 128] with row kc*128+p
        return np.ascontiguousarray(w.reshape(KC, 128, -1).transpose(1, 0, 2))

    # AllGather output rows are rank-major: row c*128 + hl*64 + d holds
    # feature (2c+hl)*64 + d; permute Wproj's contraction rows to match.
    wp_row_perm = np.empty(D, dtype=np.int64)
    for cc in range(NCORES):
        for hlhl in range(2):
            rows = np.arange(64)
            wp_row_perm[cc * 128 + hlhl * 64 + rows] = (2 * cc + hlhl) * 64 + rows

    # qkv column index for (head h, depth d, which): h*192 + d*3 + which
    d_idx = np.arange(DEPTH)
    in_maps = []
    for c in range(NCORES):
        h0, h1 = 2 * c, 2 * c + 1
        qcols = np.concatenate([h0 * 192 + 3 * d_idx, h1 * 192 + 3 * d_idx])
        kcols = qcols + 1
        vcols = qcols + 2
        in_maps.append(
            {
                "xt": xtn,
                "wq": perm_w(Wqkv[:, qcols]).astype(np_dt(qkv_dt)),
                "wk": perm_w(Wqkv[:, kcols]).astype(np_dt(qkv_dt)),
                "wv": perm_w(Wqkv[:, vcols]).astype(np_dt(qkv_dt)),
                "wp": perm_w(
                    Wproj[wp_row_perm, 128 * c : 128 * (c + 1)]
                ).astype(np_dt(proj_dt)),
                "bq": np.ascontiguousarray(bqkv[qcols]).reshape(128, 1),
                "bk": np.ascontiguousarray(bqkv[kcols]).reshape(128, 1),
                "bv": np.ascontiguousarray(bqkv[vcols]).reshape(128, 1),
                "bp": np.ascontiguousarray(
                    bproj[128 * c : 128 * (c + 1)]
                ).reshape(128, 1),
                "ident": identity,
                "ones": ones_col,
            }
        )
    return in_maps


def assemble(results):
    outT = np.concatenate([r["o"] for r in results], axis=0)  # [D, TOK]
    return np.ascontiguousarray(outT.T).reshape(B, N, D).astype(np.float32)


CONFIG = {"qkv_dt": F32R, "attn_dt": F32R, "proj_dt": BF16}

_NC_CACHE = {}


def get_nc():
    if "nc" not in _NC_CACHE:
        _NC_CACHE["nc"] = build_nc(**CONFIG)
    return _NC_CACHE["nc"]


def kernel(x, Wqkv, bqkv, Wproj, bproj):
    nc = get_nc()
    in_maps = prep_in_maps(x, Wqkv, bqkv, Wproj, bproj, **CONFIG)
    res = run_bass_kernel_spmd(nc, in_maps, list(range(NCORES)))
    return assemble(res.results)
